# revision 11
# baseline (speedup 1.0000x reference)
"""Trainium2 Bass kernel for LocalDynamicGraph edge-feature construction.

Reference computation (per batch b, point n, neighbor slot k):
    out[b, n, c,      k] = x[b, idx[b,n,k], c] - x[b, n, c]   (c in [0,64))
    out[b, n, 64 + c, k] = x[b, n, c]
Output shape (B, N, 2C, K) = (8, 16384, 128, 20) float32.

Strategy: pure data parallel, one batch per NeuronCore (B == 8 cores).
Per core the kernel is dominated by the 168 MB output write, so the design
keeps every DMA large and contiguous:

  * Neighbor rows are fetched with SWDGE dma_gather (one 256 B descriptor per
    (n, k) pair) straight from HBM into an SBUF tile laid out with
    partition = point.  The host pre-wraps the int16 index list in the
    (i%16-partition, i//16-column) layout the Q7 ucode expects, ordered so
    gather slot (t*K + k) of partition p holds neighbor k of point
    chunk_base + t*128 + p.
  * DVE computes (nbr - center) into a (128, J*2560) store tile whose free
    dim is exactly the DRAM layout of a point's (2C, K) block; ACT broadcasts
    the center half.  The store is then one fully-dense 5 MB HWDGE DMA.
"""

import sys

sys.path.insert(0, "/opt/trn_rl_repo")

import numpy as np

B, N, C, K = 8, 16384, 64, 20
P = 128          # SBUF partitions == points per point-tile
J = 4            # point-tiles per chunk
PTS = P * J      # points per chunk
NIDX = PTS * K   # gather indices per chunk
IDX_COLS = NIDX // 16
M = 2 * C * K    # 2560 output elements per point

_compiled = None


def _build(n_points: int):
    import concourse.bacc as bacc
    import concourse.mybir as mybir
    import concourse.tile as tile
    import concourse.bass as bass
    from concourse._compat import get_trn_type

    nchunk = n_points // PTS
    nc = bacc.Bacc(
        get_trn_type() or "TRN2",
        target_bir_lowering=False,
        debug=True,
        num_swdge_queues=4,
    )
    x_in = nc.dram_tensor("x", [n_points, C], mybir.dt.float32, kind="ExternalInput")
    idx_in = nc.dram_tensor(
        "idxw", [P, nchunk * IDX_COLS], mybir.dt.int16, kind="ExternalInput"
    )
    y_out = nc.dram_tensor("y", [n_points, M], mybir.dt.float32, kind="ExternalOutput")

    # point n = u*PTS + t*128 + p  ->  chunk u, free slot t, partition p
    x_r = x_in.rearrange("(u t p) c -> u p t c", t=J, p=P)
    y_r = y_out.rearrange("(u t p) m -> u p t m", t=J, p=P)

    with tile.TileContext(nc) as tc:
        with (
            tc.tile_pool(name="idxp", bufs=4) as idxp,
            tc.tile_pool(name="gp", bufs=5) as gp,
            tc.tile_pool(name="cp", bufs=2) as cp,
            tc.tile_pool(name="op", bufs=2) as op,
        ):
            for u in range(nchunk):
                idx_sb = idxp.tile([P, IDX_COLS], mybir.dt.int16)
                nc.sync.dma_start(
                    out=idx_sb[:], in_=idx_in[:, u * IDX_COLS : (u + 1) * IDX_COLS]
                )
                g = gp.tile([P, J * K, C], mybir.dt.float32)
                nc.gpsimd.dma_gather(
                    g[:],
                    x_in[:],
                    idx_sb[:],
                    NIDX,
                    NIDX,
                    C,
                    single_packet=False,
                    queue_num=u % 4,
                )
                ctr = cp.tile([P, J, C], mybir.dt.float32)
                nc.sync.dma_start(out=ctr[:], in_=x_r[u])

                o = op.tile([P, J, M], mybir.dt.float32)
                # (p, t, c, k) views of both halves of the store tile
                o_diff = o[:, :, : C * K].rearrange("p t (c k) -> p t c k", c=C)
                o_ctr = o[:, :, C * K :].rearrange("p t (c k) -> p t c k", c=C)
                g_v = g[:].rearrange("p (t k) c -> p t c k", t=J)
                c_ap = ctr[:]
                ctr_bc = bass.AP(
                    c_ap.tensor, c_ap.offset, list(c_ap.ap) + [[0, K]]
                )
                nc.vector.tensor_sub(o_diff, g_v, ctr_bc)
                nc.scalar.copy(o_ctr, ctr_bc)

                nc.sync.dma_start(out=y_r[u], in_=o[:])

    nc.compile()
    return nc


def _wrap_indices(idx_b: np.ndarray) -> np.ndarray:
    """idx_b (n_points, K) int -> (128, nchunk*IDX_COLS) int16 SBUF image."""
    n_points = idx_b.shape[0]
    blk = idx_b.reshape(-1, J, P, K)          # (u, t, p, k)
    lin = blk.transpose(0, 1, 3, 2)           # (u, t, k, p): i = (t*K+k)*P + p
    lin = lin.reshape(n_points // PTS, NIDX)  # per chunk, linear index list
    wrapped = lin.reshape(-1, IDX_COLS, 16)   # (u, col, p16)
    img = wrapped.transpose(2, 0, 1).reshape(16, -1)  # (16, nchunk*IDX_COLS)
    return np.tile(img, (8, 1)).astype(np.int16)


def kernel(x: np.ndarray, idx: np.ndarray) -> np.ndarray:
    from concourse.bass_utils import run_bass_kernel_spmd

    global _compiled
    if _compiled is None:
        _compiled = _build(N)
    nc = _compiled

    x = np.asarray(x, dtype=np.float32)
    idx = np.asarray(idx)
    in_maps = [
        {
            "x": np.ascontiguousarray(x[b]),
            "idxw": _wrap_indices(np.asarray(idx[b])),
        }
        for b in range(B)
    ]
    res = run_bass_kernel_spmd(nc, in_maps, core_ids=list(range(B)))
    out = np.stack([res.results[b]["y"].reshape(N, 2 * C, K) for b in range(B)])
    return out
